# revision 12
# baseline (speedup 1.0000x reference)
"""DeformConv3D Trainium2 Bass kernel (raw-bass, 8-core SPMD, v2).

Per core (shard = one batch x 16 z-planes = 65536 voxels, slot v at
partition v%128, column v//128):
  1. offset conv: 512 PE matmuls lhsT=[64,128] rhs=[64,3] -> psum [128,3]
     (transposed N=3 output: voxel-major offsets, no transpose dance)
  2. DVE field math: pixel coords -> (Zp,hz) (Yp,hy) (Xw,k) -> window-
     relative row index + 20 trilinear weights per voxel
     (5 x-positions x 4 zy-corners, x-selection baked into weights)
  3. idx16 build: PE double-transpose fold [128,NJ] -> [16,8NJ] int16,
     then one SBUF DMA replicates to all 8 16-partition groups
  4. gather: InstDMAGatherAnt (mlp ucode), 1 instr per 512-voxel chunk,
     512 idx x 2560B overlapping reads (stride 2048B), 4 SWDGE queues
  5. combine: DVE 5 broadcast mults (2x mode) + dz-fold; PE transposes +
     5-chain stacked-W matmul contracts (pos, c, dy); ACT copies + bias

Gather table (per core, own batch + z-window): 23 Zp-pair-planes x 2048
rows x 2KB. Row r = Zp_loc*2048 + (hz*2+hy)*512 + Yp*16 + Xw, payload
[pos4][c64][dz2][dy2] bf16; elem reads 2560B = 5 x-positions via row
overlap. Window base f(cc) = ((cc//8)-14)//2 + 7 is core-independent;
the core's z-origin is absorbed into the table slab and the per-voxel
row-offset tile zb. Indices stay in [0, 32767] (int16; z-offsets up to
+-14 voxels, ~5.5 sigma, and clamped for safety).
"""

import numpy as np
import ml_dtypes

import concourse.bass as bass
import concourse.mybir as mybir
from concourse.bass import AP
from concourse.bass_utils import run_bass_kernel_spmd
from concourse.library_config import mlp

bf16 = ml_dtypes.bfloat16
f32 = mybir.dt.float32
bft = mybir.dt.bfloat16
i32 = mybir.dt.int32
i16 = mybir.dt.int16
Alu = mybir.AluOpType
Act = mybir.ActivationFunctionType

B, CIN, COUT, D, H, W = 2, 64, 128, 64, 64, 64
NCORE = 8
SH = D // (NCORE // B)        # 16 z-planes per core
NV = SH * H * W               # 65536 voxels per core
NJ = NV // 128                # 512 slot-columns
NCHUNK = 128                  # 4 slot-cols / 512 voxels per chunk
NZP = 23                      # Zp-pair-planes in the per-core table
XQA_ROWS = NZP * 2048 + 4     # + pad rows for the 2560B overlap read
WROWS = 32768                 # gather window rows (int16 range)
NT = 128                      # xns load tiles [64, 512]
NBLK = NJ // 128              # idx fold big-blocks

# window base (in Zp planes) per chunk; core-independent by construction
BASE_F = [((cc // 8) - 14) // 2 + 7 for cc in range(NCHUNK)]

_PROGRAM = None


def _build_program(repeat=1, mode="full"):
    nc = bass.Bass(num_swdge_queues=4)

    xqa_d = nc.declare_dram_parameter("xqa", [XQA_ROWS, 1024], bft, isOutput=False)
    xns_d = nc.declare_dram_parameter("xns", [CIN, NV], f32, isOutput=False)
    btile_d = nc.declare_dram_parameter("btile", [128, NJ * 3], f32, isOutput=False)
    zb_d = nc.declare_dram_parameter("zb", [128, NJ], f32, isOutput=False)
    woff3_d = nc.declare_dram_parameter("woff3", [64, 3], f32, isOutput=False)
    w10_d = nc.declare_dram_parameter("w10", [640, 128], bft, isOutput=False)
    bconv_d = nc.declare_dram_parameter("bconv", [128, 1], f32, isOutput=False)
    ident_d = nc.declare_dram_parameter("ident", [128, 128], f32, isOutput=False)
    identb_d = nc.declare_dram_parameter("identb", [128, 128], bft, isOutput=False)
    out_d = nc.declare_dram_parameter("out", [COUT, NV], f32, isOutput=True)

    ctxs = []

    def sb(name, shape, dtype):
        cm = nc.sbuf_tensor(name, shape, dtype)
        t = cm.__enter__()
        ctxs.append(cm)
        return t

    def ps(name, shape, dtype):
        cm = nc.psum_tensor(name, shape, dtype)
        t = cm.__enter__()
        ctxs.append(cm)
        return t

    def sem(name):
        cm = nc.semaphore(name)
        s = cm.__enter__()
        ctxs.append(cm)
        return s

    # ---- SBUF ----
    btile = sb("sb_btile", [128, NJ * 3], f32)
    zb = sb("sb_zb", [128, NJ], f32)
    woff3 = sb("sb_woff3", [64, 3], f32)
    w10 = [sb(f"sb_w10_{k}", [128, 128], bft) for k in range(5)]
    bconv = sb("sb_bconv", [128, 1], f32)
    ident = sb("sb_ident", [128, 128], f32)
    identb = sb("sb_identb", [128, 128], bft)
    xcm = [sb(f"sb_xcm{i}", [64, 512], f32) for i in range(3)]
    F = sb("sb_F", [128, NJ * 3], f32)
    P = sb("sb_P", [128, NJ * 3], f32)
    Fr = sb("sb_Fr", [128, NJ * 3], f32)
    Ibig = sb("sb_Ibig", [128, NJ * 3], i32)
    tE3 = sb("sb_tE3", [128, NJ * 3], f32)
    tA = sb("sb_tA", [128, NJ], f32)
    tB = sb("sb_tB", [128, NJ], f32)
    tC = sb("sb_tC", [128, NJ], f32)
    tD = sb("sb_tD", [128, NJ], f32)
    tE = sb("sb_tE", [128, NJ], f32)
    tF2 = sb("sb_tF2", [128, NJ], f32)
    kk = sb("sb_kk", [128, NJ], f32)
    xw = sb("sb_xw", [128, NJ], f32)
    wz0 = sb("sb_wz0", [128, NJ], f32)
    wy0 = sb("sb_wy0", [128, NJ], f32)
    w4 = {zy: sb(f"sb_w4_{zy[0]}{zy[1]}", [128, NJ], f32)
          for zy in [(0, 0), (0, 1), (1, 0), (1, 1)]}
    Ism = sb("sb_Ism", [128, NJ], i32)
    Irel = sb("sb_Irel", [128, NJ], f32)
    ITs = sb("sb_ITs", [128, 128], f32)
    U20 = sb("sb_U20", [128, NJ * 20], bft)
    idx16 = sb("sb_idx16", [128, NJ * 8], i16)
    G = [sb(f"sb_G{i}", [128, 4 * 1280], bft) for i in range(3)]
    GM = [sb(f"sb_GM{i}", [128, 4 * 1280], bft) for i in range(2)]
    R1 = [sb(f"sb_R1_{i}", [128, 4 * 640], bft) for i in range(2)]
    scm = [sb(f"sb_scm{i}_{k}", [128, 512], bft)
           for i in range(2) for k in range(5)]
    ost = [sb(f"sb_ost{i}", [128, 512], f32) for i in range(2)]

    # ---- PSUM ----
    pofs = [ps("ps_pofs0", [128, 48], f32)]
    pit = [ps("ps_pit0", [128, 128], f32)]
    pix = [ps("ps_pix0", [16, 128], f32)]
    pT = [ps(f"ps_pT{i}", [128, 512], bft) for i in range(2)]
    pO = [ps(f"ps_pO{i}", [128, 512], f32) for i in range(2)]

    s_ld = sem("s_ld")
    s_xcm = sem("s_xcm")
    s_offm = sem("s_offm")
    s_F = sem("s_F")
    s_fld = sem("s_fld")
    s_it1 = sem("s_it1")
    s_it1c = sem("s_it1c")
    s_it2 = sem("s_it2")
    s_idx = sem("s_idx")
    s_idxb = sem("s_idxb")
    s_u = sem("s_u")
    s_gth = sem("s_gth")
    s_mul = sem("s_mul")
    s_fold = sem("s_fold")
    s_trp = sem("s_trp")
    s_scm = sem("s_scm")
    s_mm = sem("s_mm")
    s_act = sem("s_act")
    s_out = sem("s_out")

    def wge(eng, s, n):
        if n > 0:
            eng.wait_ge(s, n)

    with nc.Block() as block:

        # ---------------- SP: HWDGE DMA ----------------
        @block.sync
        def _(sync):
            for dst, src in [
                (btile, btile_d), (zb, zb_d), (woff3, woff3_d),
                (bconv, bconv_d), (ident, ident_d), (identb, identb_d),
            ]:
                sync.dma_start(out=dst[:], in_=src[:]).then_inc(s_ld, 16)
            for k in range(5):
                sync.dma_start(
                    out=w10[k][:], in_=w10_d[128 * k:128 * k + 128, :]
                ).then_inc(s_ld, 16)
            for i in range(NT):
                wge(sync, s_offm, 4 * (i - 2))
                sync.dma_start(
                    out=xcm[i % 3][:], in_=xns_d[:, 512 * i:512 * i + 512]
                ).then_inc(s_xcm, 16)
            # idx16 broadcast: partitions 0-15 -> 16-127 (7 groups)
            sync.wait_ge(s_idx, 4 * 8 * NBLK)
            pstep = idx16[:].ap[0][0]
            bsrc = AP(idx16[:].tensor, idx16[:].offset,
                      [[pstep, 16], [1, NJ * 8]])
            for gseg in range(1, 8):
                bdst = AP(idx16[:].tensor,
                          idx16[:].offset + 16 * gseg * pstep,
                          [[pstep, 16], [1, NJ * 8]])
                sync.dma_start(out=bdst, in_=bsrc).then_inc(s_idxb, 16)
            for r in range(repeat):
                for cc in range(NCHUNK):
                    c = r * NCHUNK + cc
                    if mode == "full":
                        wge(sync, s_act, c + 1)
                    elif mode == "nodve":
                        wge(sync, s_fold, c + 1)
                    else:
                        wge(sync, s_gth, 16 * (c + 1))
                    sync.dma_start(
                        out=out_d[:, 512 * cc:512 * cc + 512],
                        in_=ost[c % 2][:],
                    ).then_inc(s_out, 16)

        # ---------------- PE ----------------
        @block.tensor
        def _(pe):
            wge(pe, s_ld, 11 * 16)
            # phase A: offset conv (transposed, N=3)
            for t in range(NT):
                wge(pe, s_xcm, 16 * (t + 1))
                for g in range(4):
                    i = 4 * t + g
                    if i % 16 == 0:
                        wge(pe, s_F, i // 16)
                    nc.tensor.matmul(
                        out=pofs[0][:, 3 * (i % 16):3 * (i % 16) + 3],
                        lhsT=xcm[t % 3][:, 128 * g:128 * g + 128],
                        rhs=woff3[:],
                        start=True,
                        stop=True,
                    ).then_inc(s_offm, 1)

            # idx16 fold
            for bb in range(NBLK):
                wge(pe, s_fld, 1)
                wge(pe, s_it1c, bb)
                nc.tensor.transpose(
                    out=pit[0][:],
                    in_=Irel[:, 128 * bb:128 * bb + 128],
                    identity=ident[:],
                ).then_inc(s_it1, 1)
                for h in range(8):
                    m = 8 * bb + h
                    wge(pe, s_it1c, bb + 1)
                    wge(pe, s_idx, 4 * m)
                    nc.tensor.transpose(
                        out=pix[0][:],
                        in_=ITs[:, 16 * h:16 * h + 16],
                        identity=ident[:],
                    ).then_inc(s_it2, 1)

            # main loop: 20 transposes + 5-chain matmul per chunk
            for r in range(repeat):
                if mode != "full":
                    break
                for cc in range(NCHUNK):
                    c = r * NCHUNK + cc
                    wge(pe, s_fold, c + 1)
                    for tb in range(20):
                        j, k = tb % 4, tb // 4
                        wge(pe, s_scm, 5 * c + k - 1)
                        nc.tensor.transpose(
                            out=pT[k % 2][:, 128 * j:128 * j + 128],
                            in_=R1[c % 2][:, 128 * (5 * j + k):
                                          128 * (5 * j + k) + 128],
                            identity=identb[:],
                        ).then_inc(s_trp, 1)
                    wge(pe, s_scm, 5 * (c + 1))
                    wge(pe, s_act, c - 1)
                    mm = None
                    for k in range(5):
                        mm = nc.tensor.matmul(
                            out=pO[c % 2][:],
                            lhsT=w10[k][:],
                            rhs=scm[(c % 2) * 5 + k][:],
                            start=(k == 0),
                            stop=(k == 4),
                        )
                    mm.then_inc(s_mm, 1)

        # ---------------- DVE ----------------
        @block.vector
        def _(dve):
            v = nc.vector
            for t in range(NJ // 16):
                wge(dve, s_offm, 16 * (t + 1))
                v.tensor_copy(
                    out=F[:, 48 * t:48 * t + 48], in_=pofs[0][:]
                ).then_inc(s_F, 1)

            wge(dve, s_ld, 11 * 16)
            # ---- field math ----
            v.tensor_add(out=P[:], in0=F[:], in1=btile[:])
            v.tensor_scalar(out=P[:], in0=P[:], scalar1=0.0, scalar2=63.0,
                            op0=Alu.max, op1=Alu.min)
            v.tensor_copy(out=Ibig[:], in_=P[:])
            v.tensor_copy(out=Fr[:], in_=Ibig[:])
            v.tensor_tensor(out=tE3[:], in0=Fr[:], in1=P[:], op=Alu.is_gt)
            v.tensor_sub(out=Fr[:], in0=Fr[:], in1=tE3[:])   # floor(P)
            v.tensor_sub(out=P[:], in0=P[:], in1=Fr[:])      # frac
            v.tensor_copy(out=tE3[:], in_=Fr[:])
            v.tensor_copy(out=Fr[:], in_=P[:])               # Fr = frac
            v.tensor_copy(out=P[:], in_=tE3[:])              # P = floor

            def comp(tile, c_):
                return tile[:].rearrange("p (j c) -> p j c", c=3)[:, :, c_]

            ix0, iy0, iz0 = comp(P, 0), comp(P, 1), comp(P, 2)
            fx, fy, fz = comp(Fr, 0), comp(Fr, 1), comp(Fr, 2)

            def fdiv(coord, inv, q_out, rem_out, mul):
                # q = floor(coord/mul); rem = coord - mul*q
                v.tensor_scalar(out=tD[:], in0=coord, scalar1=inv,
                                scalar2=None, op0=Alu.mult)
                v.tensor_copy(out=Ism[:], in_=tD[:])
                v.tensor_copy(out=q_out, in_=Ism[:])
                v.tensor_tensor(out=tE[:], in0=q_out, in1=tD[:], op=Alu.is_gt)
                v.tensor_sub(out=q_out, in0=q_out, in1=tE[:])
                v.tensor_scalar(out=rem_out, in0=q_out, scalar1=-float(mul),
                                scalar2=None, op0=Alu.mult)
                v.tensor_add(out=rem_out, in0=rem_out, in1=coord)

            fdiv(iz0, 0.5, tA[:], tB[:], 2)     # tA=Zp, tB=hz
            fdiv(iy0, 0.5, tC[:], tF2[:], 2)    # tC=Yp, tF2=hy
            fdiv(ix0, 0.25, xw[:], kk[:], 4)    # xw=Xw, kk=k
            # Irel = Zp*2048 - zb + hz*1024 + hy*512 + Yp*16 + Xw
            v.tensor_scalar(out=Irel[:], in0=tA[:], scalar1=2048.0,
                            scalar2=None, op0=Alu.mult)
            v.tensor_sub(out=Irel[:], in0=Irel[:], in1=zb[:])
            v.tensor_scalar(out=tB[:], in0=tB[:], scalar1=1024.0,
                            scalar2=None, op0=Alu.mult)
            v.tensor_add(out=Irel[:], in0=Irel[:], in1=tB[:])
            v.tensor_scalar(out=tB[:], in0=tF2[:], scalar1=512.0,
                            scalar2=None, op0=Alu.mult)
            v.tensor_add(out=Irel[:], in0=Irel[:], in1=tB[:])
            v.tensor_scalar(out=tB[:], in0=tC[:], scalar1=16.0,
                            scalar2=None, op0=Alu.mult)
            v.tensor_add(out=Irel[:], in0=Irel[:], in1=tB[:])
            v.tensor_add(out=Irel[:], in0=Irel[:], in1=xw[:])
            v.tensor_scalar(out=Irel[:], in0=Irel[:], scalar1=0.0,
                            scalar2=32767.0, op0=Alu.max,
                            op1=Alu.min).then_inc(s_fld, 1)

            # ---- weights ----
            v.tensor_scalar(out=wz0[:], in0=fz, scalar1=-1.0, scalar2=1.0,
                            op0=Alu.mult, op1=Alu.add)
            v.tensor_scalar(out=wy0[:], in0=fy, scalar1=-1.0, scalar2=1.0,
                            op0=Alu.mult, op1=Alu.add)
            v.tensor_mul(out=w4[(0, 0)][:], in0=wz0[:], in1=wy0[:])
            v.tensor_sub(out=w4[(0, 1)][:], in0=wz0[:], in1=w4[(0, 0)][:])
            v.tensor_sub(out=w4[(1, 0)][:], in0=wy0[:], in1=w4[(0, 0)][:])
            v.tensor_sub(out=w4[(1, 1)][:], in0=fz, in1=w4[(1, 0)][:])
            uv = U20[:].rearrange("p (j s) -> p j s", s=20)
            v.tensor_scalar(out=tD[:], in0=fx, scalar1=-1.0, scalar2=1.0,
                            op0=Alu.mult, op1=Alu.add)   # tD = 1-fx
            last = None
            for pos in range(5):
                v.tensor_scalar(out=tB[:], in0=kk[:], scalar1=float(pos),
                                scalar2=None, op0=Alu.is_equal)
                v.tensor_mul(out=tB[:], in0=tB[:], in1=tD[:])
                if pos >= 1:
                    v.tensor_scalar(out=tF2[:], in0=kk[:],
                                    scalar1=float(pos - 1),
                                    scalar2=None, op0=Alu.is_equal)
                    v.tensor_mul(out=tF2[:], in0=tF2[:], in1=fx)
                    v.tensor_add(out=tB[:], in0=tB[:], in1=tF2[:])
                for zy_i, zy in enumerate([(0, 0), (0, 1), (1, 0), (1, 1)]):
                    last = v.tensor_mul(
                        out=uv[:, :, 4 * pos + zy_i], in0=tB[:],
                        in1=w4[zy][:])
            last.then_inc(s_u, 1)

            # ---- idx16 assembly ----
            for bb in range(NBLK):
                for h in range(8):
                    m = 8 * bb + h
                    wge(dve, s_it2, m + 1)
                    dst = AP(idx16[:].tensor,
                             idx16[:].offset + 1024 * bb + h,
                             [[idx16[:].ap[0][0], 16], [8, 128]])
                    v.tensor_copy(
                        out=dst, in_=pix[0][:]
                    ).then_inc(s_idx, 4)

            # ---- main loop ----
            for r in range(repeat):
                if mode == "gather":
                    break
                for cc in range(NCHUNK):
                    c = r * NCHUNK + cc
                    wge(dve, s_gth, 16 * (c + 1))
                    gt = G[c % 3]
                    gm = GM[c % 2]
                    ml = None
                    for pos in range(5):
                        g_in = AP(gt[:].tensor, gt[:].offset + 256 * pos,
                                  [gt[:].ap[0], [1280, 4], [1, 256]])
                        g_out = AP(gm[:].tensor, gm[:].offset + 256 * pos,
                                   [gm[:].ap[0], [1280, 4], [1, 256]])
                        u_in = AP(U20[:].tensor,
                                  U20[:].offset + 80 * cc + 4 * pos,
                                  [U20[:].ap[0], [20, 4], [0, 64], [1, 4]])
                        ml = nc.vector.tensor_tensor(
                            out=g_out, in0=g_in, in1=u_in, op=Alu.mult)
                    ml.then_inc(s_mul, 1)
                    if mode == "full":
                        wge(dve, s_trp, 20 * (c - 1))
                    in0 = AP(gm[:].tensor, gm[:].offset,
                             [gm[:].ap[0], [4, 1280], [1, 2]])
                    in1 = AP(gm[:].tensor, gm[:].offset + 2,
                             [gm[:].ap[0], [4, 1280], [1, 2]])
                    nc.vector.tensor_tensor(
                        out=R1[c % 2][:].rearrange("p (a b) -> p a b", b=2),
                        in0=in0, in1=in1, op=Alu.add,
                    ).then_inc(s_fold, 1)

        # ---------------- ACT ----------------
        @block.scalar
        def _(act):
            for bb in range(NBLK):
                wge(act, s_it1, bb + 1)
                wge(act, s_it2, 8 * bb)
                nc.scalar.copy(out=ITs[:], in_=pit[0][:]).then_inc(
                    s_it1c, 1)

            for r in range(repeat):
                if mode != "full":
                    break
                for cc in range(NCHUNK):
                    c = r * NCHUNK + cc
                    for k in range(5):
                        wge(act, s_trp, 20 * c + 4 * (k + 1))
                        wge(act, s_mm, c - 1)
                        nc.scalar.copy(
                            out=scm[(c % 2) * 5 + k][:],
                            in_=pT[k % 2][:],
                        ).then_inc(s_scm, 1)
                    wge(act, s_mm, c + 1)
                    wge(act, s_out, 16 * (c - 1))
                    nc.scalar.activation(
                        out=ost[c % 2][:],
                        in_=pO[c % 2][:],
                        func=Act.Identity,
                        bias=bconv[:, 0:1],
                        scale=1.0,
                    ).then_inc(s_act, 1)

        # ---------------- POOL: gathers ----------------
        @block.gpsimd
        def _(pool):
            pool.load_library(mlp)
            nreg = pool.to_reg(512)
            pool.wait_ge(s_idxb, 16 * 7)
            pool.wait_ge(s_u, 1)
            for r in range(repeat):
                for cc in range(NCHUNK):
                    c = r * NCHUNK + cc
                    if mode == "gather":
                        wge(pool, s_gth, 16 * (c - 2))
                    else:
                        wge(pool, s_mul, c - 2)
                    in_ap = AP(xqa_d[:].tensor, 1024 * 2048 * BASE_F[cc],
                               [[1024, WROWS], [1, 1280]])
                    pool.dma_gather(
                        out_ap=G[c % 3][:, 0:5120].rearrange(
                            "p (j e) -> p j e", e=1280),
                        in_ap=in_ap,
                        idxs_ap=idx16[:, 32 * cc:32 * cc + 32],
                        num_idxs=512,
                        num_idxs_reg=nreg,
                        elem_size=1280,
                        elem_step=1024,
                        queue_num=c % 4,
                    ).then_inc(s_gth, 16)

    for cm in reversed(ctxs):
        cm.__exit__(None, None, None)
    mybir.codegen_inst_isa_subclasses(nc)
    return nc


def _get_program():
    global _PROGRAM
    if _PROGRAM is None:
        _PROGRAM = _build_program()
    return _PROGRAM


def build_bench(repeat, mode="full"):
    return _build_program(repeat=repeat, mode=mode)


def _prep_inputs(x, w_off, b_off, w_conv, b_conv):
    x = np.ascontiguousarray(np.asarray(x, np.float32))
    w_off = np.asarray(w_off, np.float32)
    b_off = np.asarray(b_off, np.float32)
    w_conv = np.asarray(w_conv, np.float32)
    b_conv = np.asarray(b_conv, np.float32)

    woff3 = np.ascontiguousarray(w_off.T * 32.0)
    w10 = np.zeros((640, 128), np.float32)
    for pos in range(5):
        for c in range(64):
            for dy in range(2):
                w10[(pos * 64 + c) * 2 + dy, :] = w_conv[:, c]
    w10 = w10.astype(bf16)
    bconv = np.ascontiguousarray(b_conv.reshape(COUT, 1))
    ident = np.eye(128, dtype=np.float32)
    identb = ident.astype(bf16)

    # zext per batch: z planes [-14, 81) -> index +14; y pad +1 above
    zext = np.zeros((B, CIN, 95, H + 1, W), np.float32)
    zext[:, :, 14:14 + D, :H, :] = x
    zext = zext.astype(bf16)

    vv = np.arange(NV)
    yy = (vv // W) % H
    xx = vv % W

    in_maps = []
    for core in range(NCORE):
        b = core // (NCORE // B)
        z0 = (core % (NCORE // B)) * SH
        zpbase = z0 // 2 - 7          # Zp_glob of the core slab's Zp_loc=0

        # xqa slab: rows (Zp_loc, hz*2+hy, Yp, Xw), payload [pos,c,dz,dy]
        xqa = np.zeros((XQA_ROWS, 1024), bf16)
        # global z plane of (Zp_loc, hz, dz): 2*(Zp_loc+zpbase)+hz+dz
        # zext index = that + 14 = 2*Zp_loc + hz + dz + (z0 - 14) + 14
        zoff = z0  # zext z index base for 2*Zp_loc+hz+dz
        rows = xqa[:NZP * 2048].reshape(NZP, 4, 32, 16, 1024)
        for hz in range(2):
            for hy in range(2):
                # A[c, 2Zp+dz, 2Yp+dy, x] from zext[b,:,zoff+hz:...,hy:,:]
                A = zext[b, :, zoff + hz:zoff + hz + 2 * NZP,
                         hy:hy + 2 * 32, :]
                A = A.reshape(CIN, NZP, 2, 32, 2, 16, 4)
                # -> [Zp, Yp, Xw, pos, c, dz, dy]
                A = A.transpose(1, 3, 5, 6, 0, 2, 4)
                rows[:, 2 * hz + hy] = np.ascontiguousarray(A).reshape(
                    NZP, 32, 16, 1024)

        xns = np.ascontiguousarray(x[b, :, z0:z0 + SH].reshape(CIN, NV))
        zz = z0 + vv // (H * W)
        base = np.stack(
            [
                64.0 * xx / 63.0 - 0.5 + 32.0 * b_off[0],
                64.0 * yy / 63.0 - 0.5 + 32.0 * b_off[1],
                64.0 * zz / 63.0 - 0.5 + 32.0 * b_off[2],
            ],
            axis=1,
        ).astype(np.float32)
        btile = np.ascontiguousarray(
            base.reshape(NJ, 128, 3).transpose(1, 0, 2).reshape(128, NJ * 3)
        )
        # zb[v] = 2048 * (Zp_glob offset of the window) = 2048*(zpbase+f(cc))
        cc_of_v = vv // 512
        fcc = np.array(BASE_F, np.float32)[cc_of_v]
        zbv = 2048.0 * (zpbase + fcc)
        zb = np.ascontiguousarray(zbv.reshape(NJ, 128).T.astype(np.float32))

        in_maps.append(
            {
                "xqa": xqa,
                "xns": xns,
                "btile": btile,
                "zb": zb,
                "woff3": woff3,
                "w10": w10,
                "bconv": bconv,
                "ident": ident,
                "identb": identb,
            }
        )
    return in_maps


def _assemble(results):
    out = np.zeros((B, COUT, D, H, W), np.float32)
    for core in range(NCORE):
        b = core // (NCORE // B)
        z0 = (core % (NCORE // B)) * SH
        out[b, :, z0:z0 + SH] = results[core]["out"].reshape(COUT, SH, H, W)
    return out


def kernel(x, w_off, b_off, w_conv, b_conv):
    nc = _get_program()
    in_maps = _prep_inputs(x, w_off, b_off, w_conv, b_conv)
    res = run_bass_kernel_spmd(nc, in_maps, list(range(NCORE)))
    return _assemble(res.results)
